# revision 26
# baseline (speedup 1.0000x reference)
"""Trainium2 Bass kernel for nn_BiRNNImputerModel (bidirectional GRU imputer).

Strategy (v4 — quad-fused time segments, N=128 moving operands):
  - 8 cores: cores 0-3 run the forward GRU, cores 4-7 the backward GRU
    (backward = same program on time-reversed inputs). Within a direction,
    data-parallel over batch: 128 / 4 = 32 per core.
  - v3 ran two interleaved time-chains with N=32 moving operands; the HW
    trace showed the tensor engine 91% busy at ~26-38ns per
    LDWEIGHTS+MATMUL pair — i.e. max(ld 64cyc, N cyc) with N=32 wasting
    2/3 of PE cycles on weight loads. v4 splits each core's 511 steps
    into EIGHT warm-started time segments and runs them as TWO quads of
    4 segments marching in lockstep: every matmul's moving operand is
    [128, 4 seg * 32 batch = 128] so streaming (128cyc) now covers the
    weight load (64cyc). Same FLOPs, ~4x fewer tensor instructions.
  - Warm-start: segment q>0 starts W=15 steps early from h=0; the GRU
    error contracts ~0.64/step so the restart error at the first real
    step is ~7e-4 (measured in fp32 numpy). Warm steps consume private
    copies of their input blocks (the [iter, seg] input layout dups them
    naturally), so approximate imputation writes never pollute the
    neighboring segment's real inputs.
  - On-chip layout as v3: "transposed" [feature/H, seg*batch] so
    recurrent matmuls need no per-step transposes; each gate's 4 H-folds
    (512 = 4*128) live in the free dim of one full 2KB PSUM bank
    [128, 4 folds * 128 segbatch]. Gate nonlinearities run as one
    [128,512] ACTIVATE per gate. 1-z is computed as sigmoid(-zbank) on
    the scalar engine (ACT scale port) instead of a gpsimd op.
  - Per-fold gate biases are seeded into each bank by a K=8 "indicator"
    matmul (stationary = stacked fold-biases as bf16 hi+lo pairs,
    moving = 0/1 fold indicator) as the bank's start=True first write.
  - Input x/mask are SBUF-resident, stored as [x ; 1-m] with the
    mask-half of Wih negated and sum_f Wih_m[:,f] folded into the
    biases. Per-step imputation is one copy_predicated reading xhat
    straight from the readout PSUM (bro pre-seeded) overwriting x in
    place; the resident column block IS the gi matmul moving operand.
  - The readout matmul uses a stacked stationary [Wro.T | WoutX.T]
    producing xhat_t and this direction's partial of the final
    bidirectional readout in one accumulation.
  - The two quads interleave so each quad's ~3us gate-math tail hides
    under the other quad's ~4.5us tensor stream. Gate PSUM banks are
    SHARED between the quads (r, z, gh_n, gi_n = 4 full banks + 2
    readout bufs = 6 of 8): by the time quad B's seed for a bank enters
    the tensor FIFO, quad A's reads of it finished long ago.
  - Cross-direction sum + bout + layout fixes happen on the host; no
    cross-core communication.
"""

import os
import sys

for _p in ("/opt/trn_rl_repo", "/root/.axon_site/_ro/trn_rl_repo"):
    if os.path.isdir(_p) and _p not in sys.path:
        sys.path.insert(0, _p)

import numpy as np
import ml_dtypes

import concourse.bass as bass
import concourse.tile as tile
from concourse import mybir
from concourse.bass_utils import run_bass_kernel_spmd

BF16 = ml_dtypes.bfloat16

B, S, N, C = 128, 512, 64, 1
F = N * C          # 64
H = 512
NB = 32            # batch per core (128 / 4)
NFOLD = 4          # H / 128
NSEG = 8           # time segments per core (2 quads of 4)
QW = 4 * NB        # moving-operand width per quad = 128
WARM = 7           # warm-start steps per restarted segment
CH = 16            # iteration slots per resident-input DMA chunk
AF = mybir.ActivationFunctionType
ALU = mybir.AluOpType


def _seg_layout(n_steps):
    """Segment real-step ranges. Returns (NI, real_end[8], real_len[8]).
    Segment q's local step tau (1..NI) computes h at absolute index
    a = real_end[q] - NI + tau; steps with a <= real_end[q] - real_len[q]
    are warm-up (discarded)."""
    ni = -(-(n_steps + (NSEG - 1) * WARM) // NSEG)
    pad = NSEG * ni - (NSEG - 1) * WARM - n_steps
    real_len = [ni] + [ni - WARM] * (NSEG - 2) + [ni - WARM - pad]
    assert real_len[-1] >= 1 and ni >= WARM + 1
    real_end = np.cumsum(real_len).tolist()
    assert real_end[-1] == n_steps
    return ni, real_end, real_len


def _legalize_multiwait(nc, max_waits=1):
    """walrus in this image only encodes one sync-wait per instruction;
    hoist extra waits onto preceding NoOps."""
    n_fix = 0
    for f in nc.m.functions:
        for blk in f.blocks:
            new = []
            for ins in blk.instructions:
                si = getattr(ins, "sync_info", None)
                if si is not None and si.on_wait and len(si.on_wait) > max_waits:
                    waits = list(si.on_wait)
                    si.on_wait = waits[-max_waits:]
                    for i, w in enumerate(waits[:-max_waits]):
                        new.append(
                            mybir.InstNoOp(
                                name=f"{ins.name}-waitfix-{i}",
                                engine=ins.engine,
                                sync_info=mybir.SyncInfo(on_wait=[w], on_update=[]),
                                bass_nofuse=True,
                            )
                        )
                        n_fix += 1
                new.append(ins)
            blk.instructions[:] = new
    return n_fix


def build_nc(ni):
    """Per-core SPMD program: ni lockstep iterations of 8 segments."""
    nc = bass.Bass()
    dt = mybir.dt
    n_chunks = (ni + CH - 1) // CH

    # xm rows 0:64 = x values (slot 0 of each segment pre-imputed on host),
    # rows 64:128 = 1-m. Col block g*32:(g+1)*32 = segment g's slot.
    xm = nc.dram_tensor("xm", [128, ni, 2 * QW], dt.bfloat16, kind="ExternalInput")
    wih = nc.dram_tensor("wih", [2 * F, 3 * H], dt.bfloat16, kind="ExternalInput")
    whh = nc.dram_tensor("whh", [128, NFOLD * 3 * H], dt.bfloat16, kind="ExternalInput")
    # stacked readout: fold c -> [Wro.T fold | WoutX.T fold] = [128, 128]
    wro = nc.dram_tensor("wro", [128, NFOLD * 128], dt.bfloat16, kind="ExternalInput")
    # bhn2/bz2: rows 0:2 = bf16 hi/lo of b_hn / b_z, fold-major (col
    # s*128+p = b[s*128+p]); seed the gh_n / z banks via 4 per-fold K=2
    # matmuls each. r/gi_n biases are added post-accumulation on DVE.
    bhn2 = nc.dram_tensor("bhn2", [2, 4 * 128], dt.bfloat16, kind="ExternalInput")
    bz2 = nc.dram_tensor("bz2", [2, 4 * 128], dt.bfloat16, kind="ExternalInput")
    # bgt: fp32 broadcast bias tiles, cols 0:512 b_r, 512:1024 b_z,
    # 1024:1536 b_in; value at (p, fold*QW + j) = b[fold*128 + p]
    bgt = nc.dram_tensor("bgt", [128, 3 * 512], dt.float32, kind="ExternalInput")
    # brop: rows 0:2 = bro hi/lo (cols 0:64), zero elsewhere; ones [2, QW]
    brop = nc.dram_tensor("brop", [2, 128], dt.bfloat16, kind="ExternalInput")
    ones = nc.dram_tensor("ones", [2, QW], dt.bfloat16, kind="ExternalInput")

    op_out = nc.dram_tensor("op", [128, ni, 2 * QW], dt.float32, kind="ExternalOutput")

    with tile.TileContext(nc) as tc:
        with (
            tc.tile_pool(name="singles", bufs=1) as singles,
            tc.tile_pool(name="hist", bufs=1) as hist,
            tc.tile_pool(name="work", bufs=2) as work,
            tc.tile_pool(name="ps", bufs=1, space="PSUM") as psp,
            tc.tile_pool(name="outs", bufs=3) as outs,
        ):
            # --- load weights / biases (once) ---
            wih_sb = singles.tile([2 * F, 3 * H], dt.bfloat16)
            nc.sync.dma_start(out=wih_sb, in_=wih[:])
            whh_sb = singles.tile([128, NFOLD * 3 * H], dt.bfloat16)
            nc.sync.dma_start(out=whh_sb, in_=whh[:])
            wro_sb = singles.tile([128, NFOLD * 128], dt.bfloat16)
            nc.sync.dma_start(out=wro_sb, in_=wro[:])
            bhn2_sb = singles.tile([2, 4 * 128], dt.bfloat16)
            nc.sync.dma_start(out=bhn2_sb, in_=bhn2[:])
            bz2_sb = singles.tile([2, 4 * 128], dt.bfloat16)
            nc.sync.dma_start(out=bz2_sb, in_=bz2[:])
            bgt_sb = singles.tile([128, 3 * 512], dt.float32)
            nc.sync.dma_start(out=bgt_sb, in_=bgt[:])
            brop_sb = singles.tile([2, 128], dt.bfloat16)
            nc.sync.dma_start(out=brop_sb, in_=brop[:])
            ones_sb = singles.tile([2, QW], dt.bfloat16)
            nc.sync.dma_start(out=ones_sb, in_=ones[:])

            # --- resident input, chunked so chunk 0 gates only early iters.
            # mch duplicates the 1-m rows at partitions 0:64 because
            # copy_predicated needs out/mask/data partition-aligned. ---
            xch, mch = [], []
            for c in range(n_chunks):
                c0 = c * CH
                c1 = min(ni, c0 + CH)
                xt = singles.tile([128, c1 - c0, 2 * QW], dt.bfloat16, name=f"xch{c}")
                nc.sync.dma_start(out=xt, in_=xm[:, c0:c1, :])
                xch.append(xt)
                mt = singles.tile([F, c1 - c0, 2 * QW], dt.bfloat16, name=f"mch{c}")
                nc.sync.dma_start(out=mt, in_=xm[F : 2 * F, c0:c1, :])
                mch.append(mt)

            def xin(q, i):
                b = i - 1
                return xch[b // CH][:, b % CH, q.qi * QW : (q.qi + 1) * QW]

            def mblk(q, i):
                b = i - 1
                return mch[b // CH][:, b % CH, q.qi * QW : (q.qi + 1) * QW]

            def whh_sl(c2, gs):
                base = c2 * 3 * H + 128 * gs
                return whh_sb[:, base : base + 128]

            def wih_sl(gs):
                return wih_sb[:, 128 * gs : 128 * (gs + 1)]

            class Quad:
                pass

            quads = []
            for qi in range(2):
                q = Quad()
                q.qi = qi
                # hidden state ring: [128, parity, fold*QW]; parity = t % 2
                q.h = hist.tile([128, 2, NFOLD * QW], dt.bfloat16, name=f"h{qi}")
                nc.vector.memset(q.h[:, 0, :], 0.0)
                q.hfold = (lambda qq: lambda pv, c2:
                           qq.h[:, pv, c2 * QW : (c2 + 1) * QW])(q)
                quads.append(q)

            def emit_readout(q, i, pv, tail=False):
                """Readout of h_{i-1} (or h_ni for tail): psum <- bro +
                [Wro|WoutX]^T h (bro via K=2 seed matmul); predicated xhat
                overwrite into the x slot straight from PSUM. The SBUF copy
                + DMA for outputs is deferred to phase2 (off the critical
                path)."""
                ps_ro = psp.tile([128, QW], dt.float32, tag="ro", bufs=2,
                                 padded_shape=[128, 512], name=f"ro{q.qi}_{i}")
                nc.tensor.matmul(ps_ro, brop_sb, ones_sb, start=True, stop=False,
                                 skip_group_check=True)
                for c2 in range(NFOLD):
                    nc.tensor.matmul(ps_ro, wro_sb[:, c2 * 128 : (c2 + 1) * 128],
                                     q.hfold(pv, c2), start=False,
                                     stop=(c2 == NFOLD - 1),
                                     skip_group_check=True)
                if not tail:
                    nc.vector.copy_predicated(
                        xin(q, i)[0:F, :],
                        mblk(q, i).bitcast(mybir.dt.uint16),
                        ps_ro[0:F, :],
                    )
                return ps_ro

            def emit_out(q, i, ps_ro, tail=False):
                out_j = ni - 1 if tail else i - 2
                out_t = outs.tile([128, QW], dt.float32, tag="out_t",
                                  name=f"out{q.qi}_{i}")
                nc.scalar.activation(out=out_t, in_=ps_ro, func=AF.Copy)
                nc.sync.dma_start(
                    out=op_out[:, out_j, q.qi * QW : (q.qi + 1) * QW], in_=out_t)

            def phase1(q, i):
                """Readout + predicated-impute + the full matmul stream."""
                pv = (i - 1) % 2
                ps_ro = emit_readout(q, i, pv) if i >= 2 else None
                x_in = xin(q, i)

                # gate banks are shared between the two quads (one full 2KB
                # bank each); each gets exactly one start=True seed per use.
                rbk = psp.tile([128, NFOLD * QW], dt.float32, tag="rbank",
                               padded_shape=[128, 512], name=f"rb{q.qi}_{i}")
                zbk = psp.tile([128, NFOLD * QW], dt.float32, tag="zbank",
                               padded_shape=[128, 512], name=f"zb{q.qi}_{i}")
                ghb = psp.tile([128, NFOLD * QW], dt.float32, tag="ghbank",
                               padded_shape=[128, 512], name=f"gh{q.qi}_{i}")
                gib = psp.tile([128, NFOLD * QW], dt.float32, tag="gibank",
                               padded_shape=[128, 512], name=f"gi{q.qi}_{i}")

                def seed(bk, bsb):
                    # per-fold K=2 hi/lo bias matmuls; s==0 opens the bank
                    for s in range(NFOLD):
                        nc.tensor.matmul(
                            bk[:, s * QW : (s + 1) * QW],
                            bsb[:, s * 128 : (s + 1) * 128], ones_sb,
                            start=(s == 0), stop=False, skip_group_check=True,
                        )

                def hh(bk, gs0, last_stop, first_start=False):
                    for s in range(NFOLD):
                        reg = bk[:, s * QW : (s + 1) * QW]
                        for c2 in range(NFOLD):
                            nc.tensor.matmul(
                                reg, whh_sl(c2, gs0 + s), q.hfold(pv, c2),
                                start=(first_start and s == 0 and c2 == 0),
                                stop=(last_stop and s == NFOLD - 1
                                      and c2 == NFOLD - 1),
                                skip_group_check=True,
                            )

                def gi(bk, gs0, last_stop, first_start=False):
                    for s in range(NFOLD):
                        nc.tensor.matmul(
                            bk[:, s * QW : (s + 1) * QW], wih_sl(gs0 + s), x_in,
                            start=(first_start and s == 0),
                            stop=(last_stop and s == NFOLD - 1),
                            skip_group_check=True,
                        )

                # tensor stream: R -> Z -> gi_n -> gh_n, ordered so banks
                # close in the order phase2 consumes them. r/z/gi_n have no
                # bias seed: their first matmul opens the bank (start=True
                # clears the whole bank's has_written bits) and the bias is
                # added post-accumulation on DVE/GpSimd.
                hh(rbk, 0, last_stop=False, first_start=True)
                gi(rbk, 0, last_stop=True)
                seed(zbk, bz2_sb)
                hh(zbk, 4, last_stop=False)
                gi(zbk, 4, last_stop=True)
                gi(gib, 8, last_stop=True, first_start=True)
                seed(ghb, bhn2_sb)
                hh(ghb, 8, last_stop=True)
                q.cur = (i, ps_ro, rbk, zbk, ghb, gib)

            def phase2(q):
                """Gate nonlinearities + state update + deferred output."""
                i, ps_ro, rbk, zbk, ghb, gib = q.cur
                pv, cur = (i - 1) % 2, i % 2
                # output copy first: its ps_ro closed long ago, so it fills
                # scalar dead time before sigmoid and releases the
                # readout-bank WAR for the next seed early.
                if ps_ro is not None:
                    emit_out(q, i, ps_ro)
                    ps_ro = None
                rpre = work.tile([128, NFOLD * QW], dt.float32,
                                 tag=f"rpre{q.qi}", name=f"rp{q.qi}_{i}")
                nc.vector.tensor_tensor(rpre, rbk, bgt_sb[:, 0:512], ALU.add)
                r_t = work.tile([128, NFOLD * QW], dt.bfloat16,
                                tag=f"r_t{q.qi}", name=f"r{q.qi}_{i}")
                nc.scalar.activation(out=r_t, in_=rpre, func=AF.Sigmoid)
                # chain ops get a priority boost so the list scheduler
                # prefers them over the next stream's ops when both pend.
                # ops are emitted in data-readiness order: each engine's
                # FIFO is in-order, so a late-input op emitted early would
                # head-of-line-block ready ops behind it (z_t/omz must NOT
                # queue behind tanh, whose input nin2 arrives late).
                with tc.high_priority(offset=250):
                    z_t = work.tile([128, NFOLD * QW], dt.bfloat16,
                                    tag=f"z_t{q.qi}", name=f"z{q.qi}_{i}")
                    nc.scalar.activation(out=z_t, in_=zbk, func=AF.Sigmoid)
                    # 1-z == sigmoid(-z_pre): ACT scale port, no extra op
                    omz = work.tile([128, NFOLD * QW], dt.bfloat16,
                                    tag=f"omz{q.qi}", name=f"om{q.qi}_{i}")
                    nc.scalar.activation(out=omz, in_=zbk, func=AF.Sigmoid,
                                         scale=-1.0)
                    zh = work.tile([128, NFOLD * QW], dt.bfloat16,
                                   tag=f"zh{q.qi}", name=f"zh{q.qi}_{i}")
                    nc.gpsimd.tensor_tensor(zh, z_t, q.h[:, pv, :], ALU.mult)

                    # nin1/nin2a output bf16 so nin2 is an all-bf16 DVE op
                    # (2x_1port mode, ~414ns vs 978ns for fp32+fp32 SBUF)
                    nin2a = work.tile([128, NFOLD * QW], dt.bfloat16,
                                      tag=f"nin2a{q.qi}", name=f"na{q.qi}_{i}")
                    nc.vector.tensor_tensor(nin2a, gib, bgt_sb[:, 1024:1536],
                                            ALU.add)
                    nin1 = work.tile([128, NFOLD * QW], dt.bfloat16,
                                     tag=f"nin1{q.qi}", name=f"n1{q.qi}_{i}")
                    nc.vector.tensor_tensor(nin1, ghb, r_t, ALU.mult)
                    nin2 = work.tile([128, NFOLD * QW], dt.bfloat16,
                                     tag=f"nin2{q.qi}", name=f"n2{q.qi}_{i}")
                    nc.vector.tensor_tensor(nin2, nin1, nin2a, ALU.add)
                    n_t = work.tile([128, NFOLD * QW], dt.bfloat16,
                                    tag=f"n_t{q.qi}", name=f"n{q.qi}_{i}")
                    nc.scalar.activation(out=n_t, in_=nin2, func=AF.Tanh)

                    t3 = work.tile([128, NFOLD * QW], dt.bfloat16,
                                   tag=f"t3{q.qi}", name=f"t3{q.qi}_{i}")
                    nc.vector.tensor_tensor(t3, n_t, omz, ALU.mult)
                    nc.vector.tensor_tensor(q.h[:, cur, :], t3, zh, ALU.add)

            # software-pipelined emission: each quad's matmul stream is
            # emitted between the other quad's phase1 and phase2, so the
            # per-engine FIFO order matches the intended interleaved
            # schedule.
            q0, q1 = quads
            for it in range(1, ni + 1):
                phase1(q0, it)
                if it >= 2:
                    phase2(q1)
                phase1(q1, it)
                phase2(q0)
            phase2(q1)
            for q in quads:
                ps_ro = emit_readout(q, ni + 1, ni % 2, tail=True)
                emit_out(q, ni + 1, ps_ro, tail=True)

    _legalize_multiwait(nc)
    return nc


_NC_CACHE = {}


def _get_nc(ni):
    if ni not in _NC_CACHE:
        _NC_CACHE[ni] = build_nc(ni)
    return _NC_CACHE[ni]


def _prep_core_inputs(x2d, m2d, Wih, Whh, bih, bhh, Wro, bro, Wout_half, n_steps):
    """Per-core input map. x2d/m2d: [NB, S_loc, F] float32/bool already
    direction-ordered (time-reversed for backward cores)."""
    ni, real_end, real_len = _seg_layout(n_steps)
    Wih = np.asarray(Wih, np.float32)
    bih = np.asarray(bih, np.float32)
    bhh = np.asarray(bhh, np.float32)
    bro_f = np.asarray(bro, np.float32)

    xt = np.ascontiguousarray(x2d[:, :n_steps].transpose(2, 1, 0)).astype(np.float32)
    mt = m2d[:, :n_steps].transpose(2, 1, 0)          # [F, t, NB] bool

    # [iter, seg] input slots; warm regions get private copies naturally.
    xmf = np.empty((128, ni, NSEG * NB), np.float32)
    for g in range(NSEG):
        blocks = real_end[g] - ni + np.arange(ni)     # abs 0-based block ids
        xv = xt[:, blocks, :].copy()                  # [F, ni, NB]
        mv = mt[:, blocks, :]
        # first consumed slot pre-imputed with xhat_0 == bro (h starts at 0)
        xv[:, 0, :] = np.where(mv[:, 0, :], xv[:, 0, :], bro_f[:, None])
        cols = slice(g * NB, (g + 1) * NB)
        xmf[0:F, :, cols] = xv
        xmf[F:, :, cols] = 1.0 - mv.astype(np.float32)
    xm = xmf.astype(BF16)

    wih_t = Wih.T.copy()                               # [2F, 3H]
    wih_t[F:] = -wih_t[F:]                             # mask half negated
    wih_t = np.ascontiguousarray(wih_t).astype(BF16)
    whh_t = np.ascontiguousarray(
        np.asarray(Whh, np.float32).T.reshape(NFOLD, 128, 3 * H)
        .transpose(1, 0, 2).reshape(128, NFOLD * 3 * H)
    ).astype(BF16)
    wro_f = np.asarray(Wro, np.float32).T.reshape(NFOLD, 128, F)
    wout_f = np.asarray(Wout_half, np.float32).T.reshape(NFOLD, 128, F)
    wro_t = np.ascontiguousarray(
        np.concatenate([wro_f, wout_f], axis=2)
        .transpose(1, 0, 2).reshape(128, NFOLD * 128)
    ).astype(BF16)

    # biases with the mask-rowsum adjustment (m = 1 - inv_m)
    radj = Wih[:, F:].sum(axis=1)                      # [3H]
    bsum = bih + bhh + radj
    b_r, b_z = bsum[0:H], bsum[H : 2 * H]
    b_in = bih[2 * H :] + radj[2 * H :]
    b_hn = bhh[2 * H :]
    # hi/lo bf16 splits of b_hn / b_z, fold-major (cols = flat H index)
    def hilo(b):
        t = np.empty((2, 4 * 128), BF16)
        t[0] = b.astype(BF16)
        t[1] = (b - t[0].astype(np.float32)).astype(BF16)
        return t

    bhn2 = hilo(b_hn)
    bz2 = hilo(b_z)
    brop_f = np.zeros((2, 128), np.float32)
    brop_f[0, 0:F] = bro_f
    brop = np.empty((2, 128), BF16)
    brop[0] = brop_f[0].astype(BF16)
    brop[1] = (brop_f[0] - brop[0].astype(np.float32)).astype(BF16)

    # fp32 broadcast bias tiles for the post-accumulation adds:
    # value at (p, fold*QW + j) = b[fold*128 + p]
    def btile(b):
        t = np.ascontiguousarray(b.reshape(4, 128).T)     # [128, fold]
        return np.broadcast_to(t[:, :, None], (128, 4, QW)).reshape(128, 4 * QW)

    bgt = np.concatenate([btile(b_r), btile(b_z), btile(b_in)],
                         axis=1).astype(np.float32)

    return {
        "xm": xm, "wih": wih_t, "whh": whh_t, "wro": wro_t,
        "bhn2": bhn2, "bz2": bz2, "brop": brop, "bgt": bgt,
        "ones": np.ones((2, QW), BF16),
    }


def run_device(inputs, s_len=S, trace=False):
    """Run the 8-core SPMD kernel. Returns BassKernelResults."""
    n_steps = s_len - 1
    ni, _, _ = _seg_layout(n_steps)
    nc = _get_nc(ni)

    x2d = np.asarray(inputs["x"], np.float32).reshape(B, S, F)[:, :s_len]
    m2d = np.asarray(inputs["mask"]).reshape(B, S, F)[:, :s_len]

    in_maps = []
    for core in range(8):
        g = core % 4
        bsl = slice(NB * g, NB * (g + 1))
        if core < 4:
            im = _prep_core_inputs(
                x2d[bsl], m2d[bsl], inputs["Wih_f"], inputs["Whh_f"],
                inputs["bih_f"], inputs["bhh_f"], inputs["Wro_f"], inputs["bro_f"],
                np.asarray(inputs["Wout"])[:, :H], n_steps,
            )
        else:
            im = _prep_core_inputs(
                x2d[bsl, ::-1], m2d[bsl, ::-1], inputs["Wih_b"], inputs["Whh_b"],
                inputs["bih_b"], inputs["bhh_b"], inputs["Wro_b"], inputs["bro_b"],
                np.asarray(inputs["Wout"])[:, H:], n_steps,
            )
        in_maps.append(im)

    return run_bass_kernel_spmd(nc, in_maps, core_ids=list(range(8)), trace=trace)


def assemble(inputs, res, s_len=S):
    """Host-side gather: combine per-core outputs into full reference outputs."""
    n_steps = s_len - 1
    ni, real_end, real_len = _seg_layout(n_steps)
    bro_f = np.asarray(inputs["bro_f"], np.float32)
    bro_b = np.asarray(inputs["bro_b"], np.float32)
    bout = np.asarray(inputs["bout"], np.float32)

    xh_f = np.empty((B, s_len, F), np.float32)
    xh_b = np.empty((B, s_len, F), np.float32)
    x_hat = np.empty((B, s_len, F), np.float32)

    def unscramble(op):
        """Device op [128, ni, NSEG*NB] -> (xh_dev, pp_dev) [NB, n_steps, F]
        indexed by abs h index - 1 (a = 1..n_steps)."""
        full = np.empty((NB, n_steps, 128), np.float32)
        for g in range(NSEG):
            j0 = ni - real_len[g]
            a0 = real_end[g] - real_len[g]          # abs a = a0+1 .. real_end
            blk = op[:, j0:ni, g * NB : (g + 1) * NB]   # [128, len, NB]
            full[:, a0 : real_end[g]] = blk.transpose(2, 1, 0)
        return full[:, :, :F], full[:, :, F:]

    for g in range(4):
        bsl = slice(NB * g, NB * (g + 1))
        xf, pf = unscramble(res.results[g]["op"])
        xb, pb = unscramble(res.results[g + 4]["op"])
        xh_f[bsl, 1:] = xf
        xh_f[bsl, 0] = bro_f
        xh_b[bsl, :n_steps] = xb[:, ::-1]
        xh_b[bsl, n_steps] = bro_b
        x_hat[bsl, 1:] = pf
        x_hat[bsl, 0] = 0.0
        x_hat[bsl, :n_steps] += pb[:, ::-1]
        x_hat[bsl] += bout

    return (
        x_hat.reshape(B, s_len, N, C),
        xh_f.reshape(B, s_len, N, C),
        xh_b.reshape(B, s_len, N, C),
    )


def kernel(**inputs):
    res = run_device(inputs, s_len=S)
    return assemble(inputs, res, s_len=S)


# revision 28
# speedup vs baseline: 1.0281x; 1.0281x over previous
"""Trainium2 Bass kernel for nn_BiRNNImputerModel (bidirectional GRU imputer).

Strategy (v4 — quad-fused time segments, N=128 moving operands):
  - 8 cores: cores 0-3 run the forward GRU, cores 4-7 the backward GRU
    (backward = same program on time-reversed inputs). Within a direction,
    data-parallel over batch: 128 / 4 = 32 per core.
  - v3 ran two interleaved time-chains with N=32 moving operands; the HW
    trace showed the tensor engine 91% busy at ~26-38ns per
    LDWEIGHTS+MATMUL pair — i.e. max(ld 64cyc, N cyc) with N=32 wasting
    2/3 of PE cycles on weight loads. v4 splits each core's 511 steps
    into EIGHT warm-started time segments and runs them as TWO quads of
    4 segments marching in lockstep: every matmul's moving operand is
    [128, 4 seg * 32 batch = 128] so streaming (128cyc) now covers the
    weight load (64cyc). Same FLOPs, ~4x fewer tensor instructions.
  - Warm-start: segment q>0 starts W=15 steps early from h=0; the GRU
    error contracts ~0.64/step so the restart error at the first real
    step is ~7e-4 (measured in fp32 numpy). Warm steps consume private
    copies of their input blocks (the [iter, seg] input layout dups them
    naturally), so approximate imputation writes never pollute the
    neighboring segment's real inputs.
  - On-chip layout as v3: "transposed" [feature/H, seg*batch] so
    recurrent matmuls need no per-step transposes; each gate's 4 H-folds
    (512 = 4*128) live in the free dim of one full 2KB PSUM bank
    [128, 4 folds * 128 segbatch]. Gate nonlinearities run as one
    [128,512] ACTIVATE per gate. 1-z is computed as sigmoid(-zbank) on
    the scalar engine (ACT scale port) instead of a gpsimd op.
  - Per-fold gate biases are seeded into each bank by a K=8 "indicator"
    matmul (stationary = stacked fold-biases as bf16 hi+lo pairs,
    moving = 0/1 fold indicator) as the bank's start=True first write.
  - Input x/mask are SBUF-resident, stored as [x ; 1-m] with the
    mask-half of Wih negated and sum_f Wih_m[:,f] folded into the
    biases. Per-step imputation is one copy_predicated reading xhat
    straight from the readout PSUM (bro pre-seeded) overwriting x in
    place; the resident column block IS the gi matmul moving operand.
  - The readout matmul uses a stacked stationary [Wro.T | WoutX.T]
    producing xhat_t and this direction's partial of the final
    bidirectional readout in one accumulation.
  - The two quads interleave so each quad's ~3us gate-math tail hides
    under the other quad's ~4.5us tensor stream. Gate PSUM banks are
    SHARED between the quads (r, z, gh_n, gi_n = 4 full banks + 2
    readout bufs = 6 of 8): by the time quad B's seed for a bank enters
    the tensor FIFO, quad A's reads of it finished long ago.
  - Cross-direction sum + bout + layout fixes happen on the host; no
    cross-core communication.
"""

import os
import sys

for _p in ("/opt/trn_rl_repo", "/root/.axon_site/_ro/trn_rl_repo"):
    if os.path.isdir(_p) and _p not in sys.path:
        sys.path.insert(0, _p)

import numpy as np
import ml_dtypes

import concourse.bass as bass
import concourse.tile as tile
from concourse import mybir
from concourse.bass_utils import run_bass_kernel_spmd

BF16 = ml_dtypes.bfloat16

B, S, N, C = 128, 512, 64, 1
F = N * C          # 64
H = 512
NB = 32            # batch per core (128 / 4)
NFOLD = 4          # H / 128
NSEG = 8           # time segments per core (2 quads of 4)
QW = 4 * NB        # moving-operand width per quad = 128
WARM = 7           # warm-start steps per restarted segment
CH = 16            # iteration slots per resident-input DMA chunk
AF = mybir.ActivationFunctionType
ALU = mybir.AluOpType


def _seg_layout(n_steps):
    """Segment real-step ranges. Returns (NI, real_end[8], real_len[8]).
    Segment q's local step tau (1..NI) computes h at absolute index
    a = real_end[q] - NI + tau; steps with a <= real_end[q] - real_len[q]
    are warm-up (discarded)."""
    ni = -(-(n_steps + (NSEG - 1) * WARM) // NSEG)
    pad = NSEG * ni - (NSEG - 1) * WARM - n_steps
    real_len = [ni] + [ni - WARM] * (NSEG - 2) + [ni - WARM - pad]
    assert real_len[-1] >= 1 and ni >= WARM + 1
    real_end = np.cumsum(real_len).tolist()
    assert real_end[-1] == n_steps
    return ni, real_end, real_len


def _legalize_multiwait(nc, max_waits=1):
    """walrus in this image only encodes one sync-wait per instruction;
    hoist extra waits onto preceding NoOps."""
    n_fix = 0
    for f in nc.m.functions:
        for blk in f.blocks:
            new = []
            for ins in blk.instructions:
                si = getattr(ins, "sync_info", None)
                if si is not None and si.on_wait and len(si.on_wait) > max_waits:
                    waits = list(si.on_wait)
                    si.on_wait = waits[-max_waits:]
                    for i, w in enumerate(waits[:-max_waits]):
                        new.append(
                            mybir.InstNoOp(
                                name=f"{ins.name}-waitfix-{i}",
                                engine=ins.engine,
                                sync_info=mybir.SyncInfo(on_wait=[w], on_update=[]),
                                bass_nofuse=True,
                            )
                        )
                        n_fix += 1
                new.append(ins)
            blk.instructions[:] = new
    return n_fix


def build_nc(ni):
    """Per-core SPMD program: ni lockstep iterations of 8 segments."""
    nc = bass.Bass()
    dt = mybir.dt
    # first chunk small so the first gi matmul isn't gated on a 1MB DMA
    bounds = [0, min(4, ni)]
    while bounds[-1] < ni:
        bounds.append(min(ni, bounds[-1] + CH))

    # xm rows 0:64 = x values (slot 0 of each segment pre-imputed on host),
    # rows 64:128 = 1-m. Col block g*32:(g+1)*32 = segment g's slot.
    xm = nc.dram_tensor("xm", [128, ni, 2 * QW], dt.bfloat16, kind="ExternalInput")
    wih = nc.dram_tensor("wih", [2 * F, 3 * H], dt.bfloat16, kind="ExternalInput")
    whh = nc.dram_tensor("whh", [128, NFOLD * 3 * H], dt.bfloat16, kind="ExternalInput")
    # stacked readout: fold c -> [Wro.T fold | WoutX.T fold] = [128, 128]
    wro = nc.dram_tensor("wro", [128, NFOLD * 128], dt.bfloat16, kind="ExternalInput")
    # bhn2/bz2: rows 0:2 = bf16 hi/lo of b_hn / b_z, fold-major (col
    # s*128+p = b[s*128+p]); seed the gh_n / z banks via 4 per-fold K=2
    # matmuls each. r/gi_n biases are added post-accumulation on DVE.
    bhn2 = nc.dram_tensor("bhn2", [2, 4 * 128], dt.bfloat16, kind="ExternalInput")
    bz2 = nc.dram_tensor("bz2", [2, 4 * 128], dt.bfloat16, kind="ExternalInput")
    # bgt: fp32 broadcast bias tiles, cols 0:512 b_r, 512:1024 b_z,
    # 1024:1536 b_in; value at (p, fold*QW + j) = b[fold*128 + p]
    bgt = nc.dram_tensor("bgt", [128, 3 * 512], dt.float32, kind="ExternalInput")
    # brop: rows 0:2 = bro hi/lo (cols 0:64), zero elsewhere; ones [2, QW]
    brop = nc.dram_tensor("brop", [2, 128], dt.bfloat16, kind="ExternalInput")
    ones = nc.dram_tensor("ones", [2, QW], dt.bfloat16, kind="ExternalInput")

    op_out = nc.dram_tensor("op", [128, ni, 2 * QW], dt.float32, kind="ExternalOutput")

    with tile.TileContext(nc) as tc:
        with (
            tc.tile_pool(name="singles", bufs=1) as singles,
            tc.tile_pool(name="hist", bufs=1) as hist,
            tc.tile_pool(name="work", bufs=2) as work,
            tc.tile_pool(name="ps", bufs=1, space="PSUM") as psp,
            tc.tile_pool(name="outs", bufs=3) as outs,
        ):
            # --- load weights / biases (once) ---
            wih_sb = singles.tile([2 * F, 3 * H], dt.bfloat16)
            nc.sync.dma_start(out=wih_sb, in_=wih[:])
            whh_sb = singles.tile([128, NFOLD * 3 * H], dt.bfloat16)
            nc.sync.dma_start(out=whh_sb, in_=whh[:])
            wro_sb = singles.tile([128, NFOLD * 128], dt.bfloat16)
            nc.sync.dma_start(out=wro_sb, in_=wro[:])
            bhn2_sb = singles.tile([2, 4 * 128], dt.bfloat16)
            nc.sync.dma_start(out=bhn2_sb, in_=bhn2[:])
            bz2_sb = singles.tile([2, 4 * 128], dt.bfloat16)
            nc.sync.dma_start(out=bz2_sb, in_=bz2[:])
            bgt_sb = singles.tile([128, 3 * 512], dt.float32)
            nc.sync.dma_start(out=bgt_sb, in_=bgt[:])
            brop_sb = singles.tile([2, 128], dt.bfloat16)
            nc.sync.dma_start(out=brop_sb, in_=brop[:])
            ones_sb = singles.tile([2, QW], dt.bfloat16)
            nc.sync.dma_start(out=ones_sb, in_=ones[:])

            # --- resident input, chunked so chunk 0 gates only early iters.
            # mch duplicates the 1-m rows at partitions 0:64 because
            # copy_predicated needs out/mask/data partition-aligned. ---
            xch, mch = [], []
            for c in range(len(bounds) - 1):
                c0, c1 = bounds[c], bounds[c + 1]
                xt = singles.tile([128, c1 - c0, 2 * QW], dt.bfloat16, name=f"xch{c}")
                nc.sync.dma_start(out=xt, in_=xm[:, c0:c1, :])
                xch.append(xt)
                mt = singles.tile([F, c1 - c0, 2 * QW], dt.bfloat16, name=f"mch{c}")
                nc.sync.dma_start(out=mt, in_=xm[F : 2 * F, c0:c1, :])
                mch.append(mt)

            def _chunk(b):
                for c in range(len(bounds) - 1):
                    if b < bounds[c + 1]:
                        return c, b - bounds[c]
                raise IndexError(b)

            def xin(q, i):
                c, j = _chunk(i - 1)
                return xch[c][:, j, q.qi * QW : (q.qi + 1) * QW]

            def mblk(q, i):
                c, j = _chunk(i - 1)
                return mch[c][:, j, q.qi * QW : (q.qi + 1) * QW]

            def whh_sl(c2, gs):
                base = c2 * 3 * H + 128 * gs
                return whh_sb[:, base : base + 128]

            def wih_sl(gs):
                return wih_sb[:, 128 * gs : 128 * (gs + 1)]

            class Quad:
                pass

            quads = []
            for qi in range(2):
                q = Quad()
                q.qi = qi
                # hidden state ring: [128, parity, fold*QW]; parity = t % 2
                q.h = hist.tile([128, 2, NFOLD * QW], dt.bfloat16, name=f"h{qi}")
                nc.vector.memset(q.h[:, 0, :], 0.0)
                q.hfold = (lambda qq: lambda pv, c2:
                           qq.h[:, pv, c2 * QW : (c2 + 1) * QW])(q)
                quads.append(q)

            def emit_readout(q, i, pv, tail=False):
                """Readout of h_{i-1} (or h_ni for tail): psum <- bro +
                [Wro|WoutX]^T h (bro via K=2 seed matmul); predicated xhat
                overwrite into the x slot straight from PSUM. The SBUF copy
                + DMA for outputs is deferred to phase2 (off the critical
                path)."""
                ps_ro = psp.tile([128, QW], dt.float32, tag="ro", bufs=2,
                                 padded_shape=[128, 512], name=f"ro{q.qi}_{i}")
                nc.tensor.matmul(ps_ro, brop_sb, ones_sb, start=True, stop=False,
                                 skip_group_check=True)
                for c2 in range(NFOLD):
                    nc.tensor.matmul(ps_ro, wro_sb[:, c2 * 128 : (c2 + 1) * 128],
                                     q.hfold(pv, c2), start=False,
                                     stop=(c2 == NFOLD - 1),
                                     skip_group_check=True)
                if not tail:
                    nc.vector.copy_predicated(
                        xin(q, i)[0:F, :],
                        mblk(q, i).bitcast(mybir.dt.uint16),
                        ps_ro[0:F, :],
                    )
                return ps_ro

            def emit_out(q, i, ps_ro, tail=False):
                out_j = ni - 1 if tail else i - 2
                out_t = outs.tile([128, QW], dt.float32, tag="out_t",
                                  name=f"out{q.qi}_{i}")
                nc.scalar.activation(out=out_t, in_=ps_ro, func=AF.Copy)
                nc.sync.dma_start(
                    out=op_out[:, out_j, q.qi * QW : (q.qi + 1) * QW], in_=out_t)

            def phase1(q, i):
                """Readout + predicated-impute + the full matmul stream."""
                pv = (i - 1) % 2
                ps_ro = emit_readout(q, i, pv) if i >= 2 else None
                x_in = xin(q, i)

                # gate banks are shared between the two quads (one full 2KB
                # bank each); each gets exactly one start=True seed per use.
                rbk = psp.tile([128, NFOLD * QW], dt.float32, tag="rbank",
                               padded_shape=[128, 512], name=f"rb{q.qi}_{i}")
                zbk = psp.tile([128, NFOLD * QW], dt.float32, tag="zbank",
                               padded_shape=[128, 512], name=f"zb{q.qi}_{i}")
                ghb = psp.tile([128, NFOLD * QW], dt.float32, tag="ghbank",
                               padded_shape=[128, 512], name=f"gh{q.qi}_{i}")
                gib = psp.tile([128, NFOLD * QW], dt.float32, tag="gibank",
                               padded_shape=[128, 512], name=f"gi{q.qi}_{i}")

                def seed(bk, bsb):
                    # per-fold K=2 hi/lo bias matmuls; s==0 opens the bank
                    for s in range(NFOLD):
                        nc.tensor.matmul(
                            bk[:, s * QW : (s + 1) * QW],
                            bsb[:, s * 128 : (s + 1) * 128], ones_sb,
                            start=(s == 0), stop=False, skip_group_check=True,
                        )

                def hh(bk, gs0, last_stop, first_start=False):
                    for s in range(NFOLD):
                        reg = bk[:, s * QW : (s + 1) * QW]
                        for c2 in range(NFOLD):
                            nc.tensor.matmul(
                                reg, whh_sl(c2, gs0 + s), q.hfold(pv, c2),
                                start=(first_start and s == 0 and c2 == 0),
                                stop=(last_stop and s == NFOLD - 1
                                      and c2 == NFOLD - 1),
                                skip_group_check=True,
                            )

                def gi(bk, gs0, last_stop, first_start=False):
                    for s in range(NFOLD):
                        nc.tensor.matmul(
                            bk[:, s * QW : (s + 1) * QW], wih_sl(gs0 + s), x_in,
                            start=(first_start and s == 0),
                            stop=(last_stop and s == NFOLD - 1),
                            skip_group_check=True,
                        )

                # tensor stream: R -> Z -> gi_n -> gh_n, ordered so banks
                # close in the order phase2 consumes them. r/z/gi_n have no
                # bias seed: their first matmul opens the bank (start=True
                # clears the whole bank's has_written bits) and the bias is
                # added post-accumulation on DVE/GpSimd.
                hh(rbk, 0, last_stop=False, first_start=True)
                gi(rbk, 0, last_stop=True)
                hh(zbk, 4, last_stop=False, first_start=True)
                gi(zbk, 4, last_stop=True)
                gi(gib, 8, last_stop=True, first_start=True)
                seed(ghb, bhn2_sb)
                hh(ghb, 8, last_stop=True)
                q.cur = (i, ps_ro, rbk, zbk, ghb, gib)

            def phase2(q):
                """Gate nonlinearities + state update + deferred output."""
                i, ps_ro, rbk, zbk, ghb, gib = q.cur
                pv, cur = (i - 1) % 2, i % 2
                # output copy first: its ps_ro closed long ago, so it fills
                # scalar dead time before sigmoid and releases the
                # readout-bank WAR for the next seed early.
                if ps_ro is not None:
                    emit_out(q, i, ps_ro)
                    ps_ro = None
                rpre = work.tile([128, NFOLD * QW], dt.float32,
                                 tag=f"rpre{q.qi}", name=f"rp{q.qi}_{i}")
                nc.vector.tensor_tensor(rpre, rbk, bgt_sb[:, 0:512], ALU.add)
                r_t = work.tile([128, NFOLD * QW], dt.bfloat16,
                                tag=f"r_t{q.qi}", name=f"r{q.qi}_{i}")
                nc.scalar.activation(out=r_t, in_=rpre, func=AF.Sigmoid)
                # chain ops get a priority boost so the list scheduler
                # prefers them over the next stream's ops when both pend.
                # ops are emitted in data-readiness order: each engine's
                # FIFO is in-order, so a late-input op emitted early would
                # head-of-line-block ready ops behind it (z_t/omz must NOT
                # queue behind tanh, whose input nin2 arrives late).
                with tc.high_priority(offset=250):
                    zpre = work.tile([128, NFOLD * QW], dt.float32,
                                     tag=f"zpre{q.qi}", name=f"zp{q.qi}_{i}")
                    nc.vector.tensor_tensor(zpre, zbk, bgt_sb[:, 512:1024],
                                            ALU.add)
                    z_t = work.tile([128, NFOLD * QW], dt.bfloat16,
                                    tag=f"z_t{q.qi}", name=f"z{q.qi}_{i}")
                    nc.scalar.activation(out=z_t, in_=zpre, func=AF.Sigmoid)
                    # 1-z == sigmoid(-z_pre): ACT scale port, no extra op
                    omz = work.tile([128, NFOLD * QW], dt.bfloat16,
                                    tag=f"omz{q.qi}", name=f"om{q.qi}_{i}")
                    nc.scalar.activation(out=omz, in_=zpre, func=AF.Sigmoid,
                                         scale=-1.0)
                    zh = work.tile([128, NFOLD * QW], dt.bfloat16,
                                   tag=f"zh{q.qi}", name=f"zh{q.qi}_{i}")
                    nc.gpsimd.tensor_tensor(zh, z_t, q.h[:, pv, :], ALU.mult)

                    # nin1/nin2a output bf16 so nin2 is an all-bf16 DVE op
                    # (2x_1port mode, ~414ns vs 978ns for fp32+fp32 SBUF)
                    nin2a = work.tile([128, NFOLD * QW], dt.bfloat16,
                                      tag=f"nin2a{q.qi}", name=f"na{q.qi}_{i}")
                    nc.vector.tensor_tensor(nin2a, gib, bgt_sb[:, 1024:1536],
                                            ALU.add)
                    nin1 = work.tile([128, NFOLD * QW], dt.bfloat16,
                                     tag=f"nin1{q.qi}", name=f"n1{q.qi}_{i}")
                    nc.vector.tensor_tensor(nin1, ghb, r_t, ALU.mult)
                    nin2 = work.tile([128, NFOLD * QW], dt.bfloat16,
                                     tag=f"nin2{q.qi}", name=f"n2{q.qi}_{i}")
                    nc.vector.tensor_tensor(nin2, nin1, nin2a, ALU.add)
                    n_t = work.tile([128, NFOLD * QW], dt.bfloat16,
                                    tag=f"n_t{q.qi}", name=f"n{q.qi}_{i}")
                    nc.scalar.activation(out=n_t, in_=nin2, func=AF.Tanh)

                    t3 = work.tile([128, NFOLD * QW], dt.bfloat16,
                                   tag=f"t3{q.qi}", name=f"t3{q.qi}_{i}")
                    nc.vector.tensor_tensor(t3, n_t, omz, ALU.mult)
                    # h written in two halves: fold 0/1 land first so the
                    # next stream's hoisted ro/hh matmuls unblock earlier
                    nc.vector.tensor_tensor(q.h[:, cur, 0 : 2 * QW],
                                            t3[:, 0 : 2 * QW],
                                            zh[:, 0 : 2 * QW], ALU.add)
                    nc.vector.tensor_tensor(q.h[:, cur, 2 * QW :],
                                            t3[:, 2 * QW :],
                                            zh[:, 2 * QW :], ALU.add)

            # software-pipelined emission: each quad's matmul stream is
            # emitted between the other quad's phase1 and phase2, so the
            # per-engine FIFO order matches the intended interleaved
            # schedule.
            q0, q1 = quads
            for it in range(1, ni + 1):
                phase1(q0, it)
                if it >= 2:
                    phase2(q1)
                phase1(q1, it)
                phase2(q0)
            phase2(q1)
            for q in quads:
                ps_ro = emit_readout(q, ni + 1, ni % 2, tail=True)
                emit_out(q, ni + 1, ps_ro, tail=True)

    _legalize_multiwait(nc)
    return nc


_NC_CACHE = {}


def _get_nc(ni):
    if ni not in _NC_CACHE:
        _NC_CACHE[ni] = build_nc(ni)
    return _NC_CACHE[ni]


def _prep_core_inputs(x2d, m2d, Wih, Whh, bih, bhh, Wro, bro, Wout_half, n_steps):
    """Per-core input map. x2d/m2d: [NB, S_loc, F] float32/bool already
    direction-ordered (time-reversed for backward cores)."""
    ni, real_end, real_len = _seg_layout(n_steps)
    Wih = np.asarray(Wih, np.float32)
    bih = np.asarray(bih, np.float32)
    bhh = np.asarray(bhh, np.float32)
    bro_f = np.asarray(bro, np.float32)

    xt = np.ascontiguousarray(x2d[:, :n_steps].transpose(2, 1, 0)).astype(np.float32)
    mt = m2d[:, :n_steps].transpose(2, 1, 0)          # [F, t, NB] bool

    # [iter, seg] input slots; warm regions get private copies naturally.
    xmf = np.empty((128, ni, NSEG * NB), np.float32)
    for g in range(NSEG):
        blocks = real_end[g] - ni + np.arange(ni)     # abs 0-based block ids
        xv = xt[:, blocks, :].copy()                  # [F, ni, NB]
        mv = mt[:, blocks, :]
        # first consumed slot pre-imputed with xhat_0 == bro (h starts at 0)
        xv[:, 0, :] = np.where(mv[:, 0, :], xv[:, 0, :], bro_f[:, None])
        cols = slice(g * NB, (g + 1) * NB)
        xmf[0:F, :, cols] = xv
        xmf[F:, :, cols] = 1.0 - mv.astype(np.float32)
    xm = xmf.astype(BF16)

    wih_t = Wih.T.copy()                               # [2F, 3H]
    wih_t[F:] = -wih_t[F:]                             # mask half negated
    wih_t = np.ascontiguousarray(wih_t).astype(BF16)
    whh_t = np.ascontiguousarray(
        np.asarray(Whh, np.float32).T.reshape(NFOLD, 128, 3 * H)
        .transpose(1, 0, 2).reshape(128, NFOLD * 3 * H)
    ).astype(BF16)
    wro_f = np.asarray(Wro, np.float32).T.reshape(NFOLD, 128, F)
    wout_f = np.asarray(Wout_half, np.float32).T.reshape(NFOLD, 128, F)
    wro_t = np.ascontiguousarray(
        np.concatenate([wro_f, wout_f], axis=2)
        .transpose(1, 0, 2).reshape(128, NFOLD * 128)
    ).astype(BF16)

    # biases with the mask-rowsum adjustment (m = 1 - inv_m)
    radj = Wih[:, F:].sum(axis=1)                      # [3H]
    bsum = bih + bhh + radj
    b_r, b_z = bsum[0:H], bsum[H : 2 * H]
    b_in = bih[2 * H :] + radj[2 * H :]
    b_hn = bhh[2 * H :]
    # hi/lo bf16 splits of b_hn / b_z, fold-major (cols = flat H index)
    def hilo(b):
        t = np.empty((2, 4 * 128), BF16)
        t[0] = b.astype(BF16)
        t[1] = (b - t[0].astype(np.float32)).astype(BF16)
        return t

    bhn2 = hilo(b_hn)
    bz2 = hilo(b_z)
    brop_f = np.zeros((2, 128), np.float32)
    brop_f[0, 0:F] = bro_f
    brop = np.empty((2, 128), BF16)
    brop[0] = brop_f[0].astype(BF16)
    brop[1] = (brop_f[0] - brop[0].astype(np.float32)).astype(BF16)

    # fp32 broadcast bias tiles for the post-accumulation adds:
    # value at (p, fold*QW + j) = b[fold*128 + p]
    def btile(b):
        t = np.ascontiguousarray(b.reshape(4, 128).T)     # [128, fold]
        return np.broadcast_to(t[:, :, None], (128, 4, QW)).reshape(128, 4 * QW)

    bgt = np.concatenate([btile(b_r), btile(b_z), btile(b_in)],
                         axis=1).astype(np.float32)

    return {
        "xm": xm, "wih": wih_t, "whh": whh_t, "wro": wro_t,
        "bhn2": bhn2, "bz2": bz2, "brop": brop, "bgt": bgt,
        "ones": np.ones((2, QW), BF16),
    }


def run_device(inputs, s_len=S, trace=False):
    """Run the 8-core SPMD kernel. Returns BassKernelResults."""
    n_steps = s_len - 1
    ni, _, _ = _seg_layout(n_steps)
    nc = _get_nc(ni)

    x2d = np.asarray(inputs["x"], np.float32).reshape(B, S, F)[:, :s_len]
    m2d = np.asarray(inputs["mask"]).reshape(B, S, F)[:, :s_len]

    in_maps = []
    for core in range(8):
        g = core % 4
        bsl = slice(NB * g, NB * (g + 1))
        if core < 4:
            im = _prep_core_inputs(
                x2d[bsl], m2d[bsl], inputs["Wih_f"], inputs["Whh_f"],
                inputs["bih_f"], inputs["bhh_f"], inputs["Wro_f"], inputs["bro_f"],
                np.asarray(inputs["Wout"])[:, :H], n_steps,
            )
        else:
            im = _prep_core_inputs(
                x2d[bsl, ::-1], m2d[bsl, ::-1], inputs["Wih_b"], inputs["Whh_b"],
                inputs["bih_b"], inputs["bhh_b"], inputs["Wro_b"], inputs["bro_b"],
                np.asarray(inputs["Wout"])[:, H:], n_steps,
            )
        in_maps.append(im)

    return run_bass_kernel_spmd(nc, in_maps, core_ids=list(range(8)), trace=trace)


def assemble(inputs, res, s_len=S):
    """Host-side gather: combine per-core outputs into full reference outputs."""
    n_steps = s_len - 1
    ni, real_end, real_len = _seg_layout(n_steps)
    bro_f = np.asarray(inputs["bro_f"], np.float32)
    bro_b = np.asarray(inputs["bro_b"], np.float32)
    bout = np.asarray(inputs["bout"], np.float32)

    xh_f = np.empty((B, s_len, F), np.float32)
    xh_b = np.empty((B, s_len, F), np.float32)
    x_hat = np.empty((B, s_len, F), np.float32)

    def unscramble(op):
        """Device op [128, ni, NSEG*NB] -> (xh_dev, pp_dev) [NB, n_steps, F]
        indexed by abs h index - 1 (a = 1..n_steps)."""
        full = np.empty((NB, n_steps, 128), np.float32)
        for g in range(NSEG):
            j0 = ni - real_len[g]
            a0 = real_end[g] - real_len[g]          # abs a = a0+1 .. real_end
            blk = op[:, j0:ni, g * NB : (g + 1) * NB]   # [128, len, NB]
            full[:, a0 : real_end[g]] = blk.transpose(2, 1, 0)
        return full[:, :, :F], full[:, :, F:]

    for g in range(4):
        bsl = slice(NB * g, NB * (g + 1))
        xf, pf = unscramble(res.results[g]["op"])
        xb, pb = unscramble(res.results[g + 4]["op"])
        xh_f[bsl, 1:] = xf
        xh_f[bsl, 0] = bro_f
        xh_b[bsl, :n_steps] = xb[:, ::-1]
        xh_b[bsl, n_steps] = bro_b
        x_hat[bsl, 1:] = pf
        x_hat[bsl, 0] = 0.0
        x_hat[bsl, :n_steps] += pb[:, ::-1]
        x_hat[bsl] += bout

    return (
        x_hat.reshape(B, s_len, N, C),
        xh_f.reshape(B, s_len, N, C),
        xh_b.reshape(B, s_len, N, C),
    )


def kernel(**inputs):
    res = run_device(inputs, s_len=S)
    return assemble(inputs, res, s_len=S)


# revision 29
# speedup vs baseline: 1.1234x; 1.0926x over previous
"""Trainium2 Bass kernel for nn_BiRNNImputerModel (bidirectional GRU imputer).

Strategy (v4 — quad-fused time segments, N=128 moving operands):
  - 8 cores: cores 0-3 run the forward GRU, cores 4-7 the backward GRU
    (backward = same program on time-reversed inputs). Within a direction,
    data-parallel over batch: 128 / 4 = 32 per core.
  - v3 ran two interleaved time-chains with N=32 moving operands; the HW
    trace showed the tensor engine 91% busy at ~26-38ns per
    LDWEIGHTS+MATMUL pair — i.e. max(ld 64cyc, N cyc) with N=32 wasting
    2/3 of PE cycles on weight loads. v4 splits each core's 511 steps
    into EIGHT warm-started time segments and runs them as TWO quads of
    4 segments marching in lockstep: every matmul's moving operand is
    [128, 4 seg * 32 batch = 128] so streaming (128cyc) now covers the
    weight load (64cyc). Same FLOPs, ~4x fewer tensor instructions.
  - Warm-start: segment q>0 starts W=15 steps early from h=0; the GRU
    error contracts ~0.64/step so the restart error at the first real
    step is ~7e-4 (measured in fp32 numpy). Warm steps consume private
    copies of their input blocks (the [iter, seg] input layout dups them
    naturally), so approximate imputation writes never pollute the
    neighboring segment's real inputs.
  - On-chip layout as v3: "transposed" [feature/H, seg*batch] so
    recurrent matmuls need no per-step transposes; each gate's 4 H-folds
    (512 = 4*128) live in the free dim of one full 2KB PSUM bank
    [128, 4 folds * 128 segbatch]. Gate nonlinearities run as one
    [128,512] ACTIVATE per gate. 1-z is computed as sigmoid(-zbank) on
    the scalar engine (ACT scale port) instead of a gpsimd op.
  - Per-fold gate biases are seeded into each bank by a K=8 "indicator"
    matmul (stationary = stacked fold-biases as bf16 hi+lo pairs,
    moving = 0/1 fold indicator) as the bank's start=True first write.
  - Input x/mask are SBUF-resident, stored as [x ; 1-m] with the
    mask-half of Wih negated and sum_f Wih_m[:,f] folded into the
    biases. Per-step imputation is one copy_predicated reading xhat
    straight from the readout PSUM (bro pre-seeded) overwriting x in
    place; the resident column block IS the gi matmul moving operand.
  - The readout matmul uses a stacked stationary [Wro.T | WoutX.T]
    producing xhat_t and this direction's partial of the final
    bidirectional readout in one accumulation.
  - The two quads interleave so each quad's ~3us gate-math tail hides
    under the other quad's ~4.5us tensor stream. Gate PSUM banks are
    SHARED between the quads (r, z, gh_n, gi_n = 4 full banks + 2
    readout bufs = 6 of 8): by the time quad B's seed for a bank enters
    the tensor FIFO, quad A's reads of it finished long ago.
  - Cross-direction sum + bout + layout fixes happen on the host; no
    cross-core communication.
"""

import os
import sys

for _p in ("/opt/trn_rl_repo", "/root/.axon_site/_ro/trn_rl_repo"):
    if os.path.isdir(_p) and _p not in sys.path:
        sys.path.insert(0, _p)

import numpy as np
import ml_dtypes

import concourse.bass as bass
import concourse.tile as tile
from concourse import mybir
from concourse.bass_utils import run_bass_kernel_spmd

BF16 = ml_dtypes.bfloat16

B, S, N, C = 128, 512, 64, 1
F = N * C          # 64
H = 512
NB = 32            # batch per core (128 / 4)
NFOLD = 4          # H / 128
NSEG = 8           # time segments per core (2 quads of 4)
QW = 4 * NB        # moving-operand width per quad = 128
WARM = 7           # warm-start steps per restarted segment
CH = 16            # iteration slots per resident-input DMA chunk
AF = mybir.ActivationFunctionType
ALU = mybir.AluOpType


def _seg_layout(n_steps):
    """Segment real-step ranges. Returns (NI, real_end[8], real_len[8]).
    Segment q's local step tau (1..NI) computes h at absolute index
    a = real_end[q] - NI + tau; steps with a <= real_end[q] - real_len[q]
    are warm-up (discarded)."""
    ni = -(-(n_steps + (NSEG - 1) * WARM) // NSEG)
    pad = NSEG * ni - (NSEG - 1) * WARM - n_steps
    real_len = [ni] + [ni - WARM] * (NSEG - 2) + [ni - WARM - pad]
    assert real_len[-1] >= 1 and ni >= WARM + 1
    real_end = np.cumsum(real_len).tolist()
    assert real_end[-1] == n_steps
    return ni, real_end, real_len


def _legalize_multiwait(nc, max_waits=1):
    """walrus in this image only encodes one sync-wait per instruction;
    hoist extra waits onto preceding NoOps."""
    n_fix = 0
    for f in nc.m.functions:
        for blk in f.blocks:
            new = []
            for ins in blk.instructions:
                si = getattr(ins, "sync_info", None)
                if si is not None and si.on_wait and len(si.on_wait) > max_waits:
                    waits = list(si.on_wait)
                    si.on_wait = waits[-max_waits:]
                    for i, w in enumerate(waits[:-max_waits]):
                        new.append(
                            mybir.InstNoOp(
                                name=f"{ins.name}-waitfix-{i}",
                                engine=ins.engine,
                                sync_info=mybir.SyncInfo(on_wait=[w], on_update=[]),
                                bass_nofuse=True,
                            )
                        )
                        n_fix += 1
                new.append(ins)
            blk.instructions[:] = new
    return n_fix


def build_nc(ni):
    """Per-core SPMD program: ni lockstep iterations of 8 segments."""
    nc = bass.Bass()
    dt = mybir.dt
    # first chunk small so the first gi matmul isn't gated on a 1MB DMA
    bounds = [0, min(4, ni)]
    while bounds[-1] < ni:
        bounds.append(min(ni, bounds[-1] + CH))

    # xm rows 0:64 = x values (slot 0 of each segment pre-imputed on host),
    # rows 64:128 = 1-m. Col block g*32:(g+1)*32 = segment g's slot.
    xm = nc.dram_tensor("xm", [128, ni, 2 * QW], dt.bfloat16, kind="ExternalInput")
    wih = nc.dram_tensor("wih", [2 * F, 3 * H], dt.bfloat16, kind="ExternalInput")
    whh = nc.dram_tensor("whh", [128, NFOLD * 3 * H], dt.bfloat16, kind="ExternalInput")
    # stacked readout: fold c -> [Wro.T fold | WoutX.T fold] = [128, 128]
    wro = nc.dram_tensor("wro", [128, NFOLD * 128], dt.bfloat16, kind="ExternalInput")
    # bhn2/bz2: rows 0:2 = bf16 hi/lo of b_hn / b_z, fold-major (col
    # s*128+p = b[s*128+p]); seed the gh_n / z banks via 4 per-fold K=2
    # matmuls each. r/gi_n biases are added post-accumulation on DVE.
    bhn2 = nc.dram_tensor("bhn2", [2, 4 * 128], dt.bfloat16, kind="ExternalInput")
    bz2 = nc.dram_tensor("bz2", [2, 4 * 128], dt.bfloat16, kind="ExternalInput")
    # bgt: fp32 broadcast bias tiles, cols 0:512 b_r, 512:1024 b_z,
    # 1024:1536 b_in; value at (p, fold*QW + j) = b[fold*128 + p]
    bgt = nc.dram_tensor("bgt", [128, 3 * 512], dt.float32, kind="ExternalInput")
    # brop: rows 0:2 = bro hi/lo (cols 0:64), zero elsewhere; ones [2, QW]
    brop = nc.dram_tensor("brop", [2, 128], dt.bfloat16, kind="ExternalInput")
    ones = nc.dram_tensor("ones", [2, QW], dt.bfloat16, kind="ExternalInput")

    op_out = nc.dram_tensor("op", [128, ni, 2 * QW], dt.float32, kind="ExternalOutput")

    with tile.TileContext(nc) as tc:
        with (
            tc.tile_pool(name="singles", bufs=1) as singles,
            tc.tile_pool(name="hist", bufs=1) as hist,
            tc.tile_pool(name="work", bufs=2) as work,
            tc.tile_pool(name="ps", bufs=1, space="PSUM") as psp,
            tc.tile_pool(name="outs", bufs=3) as outs,
        ):
            # --- load weights / biases (once) ---
            wih_sb = singles.tile([2 * F, 3 * H], dt.bfloat16)
            nc.sync.dma_start(out=wih_sb, in_=wih[:])
            whh_sb = singles.tile([128, NFOLD * 3 * H], dt.bfloat16)
            nc.sync.dma_start(out=whh_sb, in_=whh[:])
            wro_sb = singles.tile([128, NFOLD * 128], dt.bfloat16)
            nc.sync.dma_start(out=wro_sb, in_=wro[:])
            bhn2_sb = singles.tile([2, 4 * 128], dt.bfloat16)
            nc.sync.dma_start(out=bhn2_sb, in_=bhn2[:])
            bz2_sb = singles.tile([2, 4 * 128], dt.bfloat16)
            nc.sync.dma_start(out=bz2_sb, in_=bz2[:])
            bgt_sb = singles.tile([128, 3 * 512], dt.float32)
            nc.sync.dma_start(out=bgt_sb, in_=bgt[:])
            brop_sb = singles.tile([2, 128], dt.bfloat16)
            nc.sync.dma_start(out=brop_sb, in_=brop[:])
            ones_sb = singles.tile([2, QW], dt.bfloat16)
            nc.sync.dma_start(out=ones_sb, in_=ones[:])

            # --- resident input, chunked so chunk 0 gates only early iters.
            # mch duplicates the 1-m rows at partitions 0:64 because
            # copy_predicated needs out/mask/data partition-aligned. ---
            xch, mch = [], []
            for c in range(len(bounds) - 1):
                c0, c1 = bounds[c], bounds[c + 1]
                xt = singles.tile([128, c1 - c0, 2 * QW], dt.bfloat16, name=f"xch{c}")
                nc.sync.dma_start(out=xt, in_=xm[:, c0:c1, :])
                xch.append(xt)
                mt = singles.tile([F, c1 - c0, 2 * QW], dt.bfloat16, name=f"mch{c}")
                nc.sync.dma_start(out=mt, in_=xm[F : 2 * F, c0:c1, :])
                mch.append(mt)

            def _chunk(b):
                for c in range(len(bounds) - 1):
                    if b < bounds[c + 1]:
                        return c, b - bounds[c]
                raise IndexError(b)

            def xin(q, i):
                c, j = _chunk(i - 1)
                return xch[c][:, j, q.qi * QW : (q.qi + 1) * QW]

            def mblk(q, i):
                c, j = _chunk(i - 1)
                return mch[c][:, j, q.qi * QW : (q.qi + 1) * QW]

            def whh_sl(c2, gs):
                base = c2 * 3 * H + 128 * gs
                return whh_sb[:, base : base + 128]

            def wih_sl(gs):
                return wih_sb[:, 128 * gs : 128 * (gs + 1)]

            class Quad:
                pass

            quads = []
            for qi in range(2):
                q = Quad()
                q.qi = qi
                # hidden state ring: [128, parity, fold*QW]; parity = t % 2
                q.h = hist.tile([128, 2, NFOLD * QW], dt.bfloat16, name=f"h{qi}")
                nc.vector.memset(q.h[:, 0, :], 0.0)
                q.hfold = (lambda qq: lambda pv, c2:
                           qq.h[:, pv, c2 * QW : (c2 + 1) * QW])(q)
                quads.append(q)

            def emit_readout(q, i, pv, tail=False):
                """Readout of h_{i-1} (or h_ni for tail): psum <- bro +
                [Wro|WoutX]^T h (bro via K=2 seed matmul); predicated xhat
                overwrite into the x slot straight from PSUM. The SBUF copy
                + DMA for outputs is deferred to phase2 (off the critical
                path)."""
                ps_ro = psp.tile([128, QW], dt.float32, tag="ro", bufs=2,
                                 padded_shape=[128, 512], name=f"ro{q.qi}_{i}")
                nc.tensor.matmul(ps_ro, brop_sb, ones_sb, start=True, stop=False,
                                 skip_group_check=True)
                for c2 in range(NFOLD):
                    nc.tensor.matmul(ps_ro, wro_sb[:, c2 * 128 : (c2 + 1) * 128],
                                     q.hfold(pv, c2), start=False,
                                     stop=(c2 == NFOLD - 1),
                                     skip_group_check=True)
                if not tail:
                    nc.vector.copy_predicated(
                        xin(q, i)[0:F, :],
                        mblk(q, i).bitcast(mybir.dt.uint16),
                        ps_ro[0:F, :],
                    )
                return ps_ro

            def emit_out(q, i, ps_ro, tail=False):
                out_j = ni - 1 if tail else i - 2
                out_t = outs.tile([128, QW], dt.float32, tag="out_t",
                                  name=f"out{q.qi}_{i}")
                nc.scalar.activation(out=out_t, in_=ps_ro, func=AF.Copy)
                nc.sync.dma_start(
                    out=op_out[:, out_j, q.qi * QW : (q.qi + 1) * QW], in_=out_t)

            def phase1(q, i):
                """Readout + predicated-impute + the full matmul stream."""
                pv = (i - 1) % 2
                ps_ro = emit_readout(q, i, pv) if i >= 2 else None
                x_in = xin(q, i)

                # gate banks are shared between the two quads (one full 2KB
                # bank each); each gets exactly one start=True seed per use.
                rbk = psp.tile([128, NFOLD * QW], dt.float32, tag="rbank",
                               padded_shape=[128, 512], name=f"rb{q.qi}_{i}")
                zbk = psp.tile([128, NFOLD * QW], dt.float32, tag="zbank",
                               padded_shape=[128, 512], name=f"zb{q.qi}_{i}")
                ghb = psp.tile([128, NFOLD * QW], dt.float32, tag="ghbank",
                               padded_shape=[128, 512], name=f"gh{q.qi}_{i}")
                gib = psp.tile([128, NFOLD * QW], dt.float32, tag="gibank",
                               padded_shape=[128, 512], name=f"gi{q.qi}_{i}")

                def seed(bk, bsb):
                    # per-fold K=2 hi/lo bias matmuls; s==0 opens the bank
                    for s in range(NFOLD):
                        nc.tensor.matmul(
                            bk[:, s * QW : (s + 1) * QW],
                            bsb[:, s * 128 : (s + 1) * 128], ones_sb,
                            start=(s == 0), stop=False, skip_group_check=True,
                        )

                def hh(bk, gs0, last_stop, first_start=False):
                    for s in range(NFOLD):
                        reg = bk[:, s * QW : (s + 1) * QW]
                        for c2 in range(NFOLD):
                            nc.tensor.matmul(
                                reg, whh_sl(c2, gs0 + s), q.hfold(pv, c2),
                                start=(first_start and s == 0 and c2 == 0),
                                stop=(last_stop and s == NFOLD - 1
                                      and c2 == NFOLD - 1),
                                skip_group_check=True,
                            )

                def gi(bk, gs0, last_stop, first_start=False):
                    for s in range(NFOLD):
                        nc.tensor.matmul(
                            bk[:, s * QW : (s + 1) * QW], wih_sl(gs0 + s), x_in,
                            start=(first_start and s == 0),
                            stop=(last_stop and s == NFOLD - 1),
                            skip_group_check=True,
                        )

                # tensor stream: R -> Z -> gi_n -> gh_n, ordered so banks
                # close in the order phase2 consumes them. r/z/gi_n have no
                # bias seed: their first matmul opens the bank (start=True
                # clears the whole bank's has_written bits) and the bias is
                # added post-accumulation on DVE/GpSimd.
                hh(rbk, 0, last_stop=False, first_start=True)
                gi(rbk, 0, last_stop=True)
                hh(zbk, 4, last_stop=False, first_start=True)
                gi(zbk, 4, last_stop=True)
                gi(gib, 8, last_stop=True, first_start=True)
                seed(ghb, bhn2_sb)
                hh(ghb, 8, last_stop=True)
                q.cur = (i, ps_ro, rbk, zbk, ghb, gib)

            def phase2(q):
                """Gate nonlinearities + state update + deferred output."""
                i, ps_ro, rbk, zbk, ghb, gib = q.cur
                pv, cur = (i - 1) % 2, i % 2
                # output copy first: its ps_ro closed long ago, so it fills
                # scalar dead time before sigmoid and releases the
                # readout-bank WAR for the next seed early.
                if ps_ro is not None:
                    emit_out(q, i, ps_ro)
                    ps_ro = None
                rpre = work.tile([128, NFOLD * QW], dt.float32,
                                 tag=f"rpre{q.qi}", name=f"rp{q.qi}_{i}")
                nc.vector.tensor_tensor(rpre, rbk, bgt_sb[:, 0:512], ALU.add)
                r_t = work.tile([128, NFOLD * QW], dt.bfloat16,
                                tag=f"r_t{q.qi}", name=f"r{q.qi}_{i}")
                nc.scalar.activation(out=r_t, in_=rpre, func=AF.Sigmoid)
                # chain ops get a priority boost so the list scheduler
                # prefers them over the next stream's ops when both pend.
                # ops are emitted in data-readiness order: each engine's
                # FIFO is in-order, so a late-input op emitted early would
                # head-of-line-block ready ops behind it (z_t/omz must NOT
                # queue behind tanh, whose input nin2 arrives late).
                with tc.high_priority(offset=250):
                    zpre = work.tile([128, NFOLD * QW], dt.float32,
                                     tag=f"zpre{q.qi}", name=f"zp{q.qi}_{i}")
                    nc.vector.tensor_tensor(zpre, zbk, bgt_sb[:, 512:1024],
                                            ALU.add)
                    z_t = work.tile([128, NFOLD * QW], dt.bfloat16,
                                    tag=f"z_t{q.qi}", name=f"z{q.qi}_{i}")
                    nc.scalar.activation(out=z_t, in_=zpre, func=AF.Sigmoid)
                    # 1-z == sigmoid(-z_pre): ACT scale port, no extra op
                    omz = work.tile([128, NFOLD * QW], dt.bfloat16,
                                    tag=f"omz{q.qi}", name=f"om{q.qi}_{i}")
                    nc.scalar.activation(out=omz, in_=zpre, func=AF.Sigmoid,
                                         scale=-1.0)
                    zh = work.tile([128, NFOLD * QW], dt.bfloat16,
                                   tag=f"zh{q.qi}", name=f"zh{q.qi}_{i}")
                    nc.gpsimd.tensor_tensor(zh, z_t, q.h[:, pv, :], ALU.mult)

                    # nin1/nin2a output bf16 so nin2 is an all-bf16 DVE op
                    # (2x_1port mode, ~414ns vs 978ns for fp32+fp32 SBUF)
                    nin2a = work.tile([128, NFOLD * QW], dt.bfloat16,
                                      tag=f"nin2a{q.qi}", name=f"na{q.qi}_{i}")
                    nc.vector.tensor_tensor(nin2a, gib, bgt_sb[:, 1024:1536],
                                            ALU.add)
                    nin1 = work.tile([128, NFOLD * QW], dt.bfloat16,
                                     tag=f"nin1{q.qi}", name=f"n1{q.qi}_{i}")
                    nc.vector.tensor_tensor(nin1, ghb, r_t, ALU.mult)
                    nin2 = work.tile([128, NFOLD * QW], dt.bfloat16,
                                     tag=f"nin2{q.qi}", name=f"n2{q.qi}_{i}")
                    nc.vector.tensor_tensor(nin2, nin1, nin2a, ALU.add)
                    n_t = work.tile([128, NFOLD * QW], dt.bfloat16,
                                    tag=f"n_t{q.qi}", name=f"n{q.qi}_{i}")
                    nc.scalar.activation(out=n_t, in_=nin2, func=AF.Tanh)

                    t3 = work.tile([128, NFOLD * QW], dt.bfloat16,
                                   tag=f"t3{q.qi}", name=f"t3{q.qi}_{i}")
                    nc.vector.tensor_tensor(t3, n_t, omz, ALU.mult)
                    nc.vector.tensor_tensor(q.h[:, cur, :], t3, zh, ALU.add)

            # software-pipelined emission: each quad's matmul stream is
            # emitted between the other quad's phase1 and phase2, so the
            # per-engine FIFO order matches the intended interleaved
            # schedule.
            q0, q1 = quads
            for it in range(1, ni + 1):
                phase1(q0, it)
                if it >= 2:
                    phase2(q1)
                phase1(q1, it)
                phase2(q0)
            phase2(q1)
            for q in quads:
                ps_ro = emit_readout(q, ni + 1, ni % 2, tail=True)
                emit_out(q, ni + 1, ps_ro, tail=True)

    _legalize_multiwait(nc)
    return nc


_NC_CACHE = {}


def _get_nc(ni):
    if ni not in _NC_CACHE:
        _NC_CACHE[ni] = build_nc(ni)
    return _NC_CACHE[ni]


def _prep_core_inputs(x2d, m2d, Wih, Whh, bih, bhh, Wro, bro, Wout_half, n_steps):
    """Per-core input map. x2d/m2d: [NB, S_loc, F] float32/bool already
    direction-ordered (time-reversed for backward cores)."""
    ni, real_end, real_len = _seg_layout(n_steps)
    Wih = np.asarray(Wih, np.float32)
    bih = np.asarray(bih, np.float32)
    bhh = np.asarray(bhh, np.float32)
    bro_f = np.asarray(bro, np.float32)

    xt = np.ascontiguousarray(x2d[:, :n_steps].transpose(2, 1, 0)).astype(np.float32)
    mt = m2d[:, :n_steps].transpose(2, 1, 0)          # [F, t, NB] bool

    # [iter, seg] input slots; warm regions get private copies naturally.
    xmf = np.empty((128, ni, NSEG * NB), np.float32)
    for g in range(NSEG):
        blocks = real_end[g] - ni + np.arange(ni)     # abs 0-based block ids
        xv = xt[:, blocks, :].copy()                  # [F, ni, NB]
        mv = mt[:, blocks, :]
        # first consumed slot pre-imputed with xhat_0 == bro (h starts at 0)
        xv[:, 0, :] = np.where(mv[:, 0, :], xv[:, 0, :], bro_f[:, None])
        cols = slice(g * NB, (g + 1) * NB)
        xmf[0:F, :, cols] = xv
        xmf[F:, :, cols] = 1.0 - mv.astype(np.float32)
    xm = xmf.astype(BF16)

    wih_t = Wih.T.copy()                               # [2F, 3H]
    wih_t[F:] = -wih_t[F:]                             # mask half negated
    wih_t = np.ascontiguousarray(wih_t).astype(BF16)
    whh_t = np.ascontiguousarray(
        np.asarray(Whh, np.float32).T.reshape(NFOLD, 128, 3 * H)
        .transpose(1, 0, 2).reshape(128, NFOLD * 3 * H)
    ).astype(BF16)
    wro_f = np.asarray(Wro, np.float32).T.reshape(NFOLD, 128, F)
    wout_f = np.asarray(Wout_half, np.float32).T.reshape(NFOLD, 128, F)
    wro_t = np.ascontiguousarray(
        np.concatenate([wro_f, wout_f], axis=2)
        .transpose(1, 0, 2).reshape(128, NFOLD * 128)
    ).astype(BF16)

    # biases with the mask-rowsum adjustment (m = 1 - inv_m)
    radj = Wih[:, F:].sum(axis=1)                      # [3H]
    bsum = bih + bhh + radj
    b_r, b_z = bsum[0:H], bsum[H : 2 * H]
    b_in = bih[2 * H :] + radj[2 * H :]
    b_hn = bhh[2 * H :]
    # hi/lo bf16 splits of b_hn / b_z, fold-major (cols = flat H index)
    def hilo(b):
        t = np.empty((2, 4 * 128), BF16)
        t[0] = b.astype(BF16)
        t[1] = (b - t[0].astype(np.float32)).astype(BF16)
        return t

    bhn2 = hilo(b_hn)
    bz2 = hilo(b_z)
    brop_f = np.zeros((2, 128), np.float32)
    brop_f[0, 0:F] = bro_f
    brop = np.empty((2, 128), BF16)
    brop[0] = brop_f[0].astype(BF16)
    brop[1] = (brop_f[0] - brop[0].astype(np.float32)).astype(BF16)

    # fp32 broadcast bias tiles for the post-accumulation adds:
    # value at (p, fold*QW + j) = b[fold*128 + p]
    def btile(b):
        t = np.ascontiguousarray(b.reshape(4, 128).T)     # [128, fold]
        return np.broadcast_to(t[:, :, None], (128, 4, QW)).reshape(128, 4 * QW)

    bgt = np.concatenate([btile(b_r), btile(b_z), btile(b_in)],
                         axis=1).astype(np.float32)

    return {
        "xm": xm, "wih": wih_t, "whh": whh_t, "wro": wro_t,
        "bhn2": bhn2, "bz2": bz2, "brop": brop, "bgt": bgt,
        "ones": np.ones((2, QW), BF16),
    }


def run_device(inputs, s_len=S, trace=False):
    """Run the 8-core SPMD kernel. Returns BassKernelResults."""
    n_steps = s_len - 1
    ni, _, _ = _seg_layout(n_steps)
    nc = _get_nc(ni)

    x2d = np.asarray(inputs["x"], np.float32).reshape(B, S, F)[:, :s_len]
    m2d = np.asarray(inputs["mask"]).reshape(B, S, F)[:, :s_len]

    in_maps = []
    for core in range(8):
        g = core % 4
        bsl = slice(NB * g, NB * (g + 1))
        if core < 4:
            im = _prep_core_inputs(
                x2d[bsl], m2d[bsl], inputs["Wih_f"], inputs["Whh_f"],
                inputs["bih_f"], inputs["bhh_f"], inputs["Wro_f"], inputs["bro_f"],
                np.asarray(inputs["Wout"])[:, :H], n_steps,
            )
        else:
            im = _prep_core_inputs(
                x2d[bsl, ::-1], m2d[bsl, ::-1], inputs["Wih_b"], inputs["Whh_b"],
                inputs["bih_b"], inputs["bhh_b"], inputs["Wro_b"], inputs["bro_b"],
                np.asarray(inputs["Wout"])[:, H:], n_steps,
            )
        in_maps.append(im)

    return run_bass_kernel_spmd(nc, in_maps, core_ids=list(range(8)), trace=trace)


def assemble(inputs, res, s_len=S):
    """Host-side gather: combine per-core outputs into full reference outputs."""
    n_steps = s_len - 1
    ni, real_end, real_len = _seg_layout(n_steps)
    bro_f = np.asarray(inputs["bro_f"], np.float32)
    bro_b = np.asarray(inputs["bro_b"], np.float32)
    bout = np.asarray(inputs["bout"], np.float32)

    xh_f = np.empty((B, s_len, F), np.float32)
    xh_b = np.empty((B, s_len, F), np.float32)
    x_hat = np.empty((B, s_len, F), np.float32)

    def unscramble(op):
        """Device op [128, ni, NSEG*NB] -> (xh_dev, pp_dev) [NB, n_steps, F]
        indexed by abs h index - 1 (a = 1..n_steps)."""
        full = np.empty((NB, n_steps, 128), np.float32)
        for g in range(NSEG):
            j0 = ni - real_len[g]
            a0 = real_end[g] - real_len[g]          # abs a = a0+1 .. real_end
            blk = op[:, j0:ni, g * NB : (g + 1) * NB]   # [128, len, NB]
            full[:, a0 : real_end[g]] = blk.transpose(2, 1, 0)
        return full[:, :, :F], full[:, :, F:]

    for g in range(4):
        bsl = slice(NB * g, NB * (g + 1))
        xf, pf = unscramble(res.results[g]["op"])
        xb, pb = unscramble(res.results[g + 4]["op"])
        xh_f[bsl, 1:] = xf
        xh_f[bsl, 0] = bro_f
        xh_b[bsl, :n_steps] = xb[:, ::-1]
        xh_b[bsl, n_steps] = bro_b
        x_hat[bsl, 1:] = pf
        x_hat[bsl, 0] = 0.0
        x_hat[bsl, :n_steps] += pb[:, ::-1]
        x_hat[bsl] += bout

    return (
        x_hat.reshape(B, s_len, N, C),
        xh_f.reshape(B, s_len, N, C),
        xh_b.reshape(B, s_len, N, C),
    )


def kernel(**inputs):
    res = run_device(inputs, s_len=S)
    return assemble(inputs, res, s_len=S)
